# revision 13
# baseline (speedup 1.0000x reference)
"""Trainium2 Bass kernel for nn_Attention_326417514823.

Per-batch computation (B=8, N=2048, D=256), one batch per NeuronCore:
    S = Q @ K.T / sqrt(D)                  (N x N)
    S[q, :] = -1e9 where mask[q] == 0      (row masking by query index)
    A = softmax(S, axis=0)                 (normalize over q, per column k)
    A[q, :] = 0 where mask[q] == 0
    O = A @ V                              (N x D)

Key restructuring vs a dense kernel:

1. HOST-SIDE QUERY COMPACTION. The softmax axis is q, and masked queries
   contribute nothing: their output rows are zero and they are excluded
   from every softmax sum. The host packs the first <=1024 unmasked
   queries into a fixed [256, 1024] device tile (pad columns are zero ->
   scores 0 -> E=1, subtracted out of the normalizer via a host-provided
   per-k correction).  Overflow queries (n_u > 1024, a ~2% tail) are
   handled exactly on the host using the device-returned normalizers
   c[k]: O_excess = (exp(S_excess)/c).T @ V.  This halves all on-device
   work (PE, exp, DMA).

2. TRANSPOSED LAYOUT. ST[k, q] = KT.T @ QT with d on partitions, so the
   softmax reduction runs along the free axis and neither matmul needs an
   on-chip transpose:
     E[k, q]  = exp(ST/16)                   (fp16, ScalarE, fused accum c)
     W[k, :]  = V[k, :] * (1/c[k])           (fp16, DVE)
     OT[d, q] = sum_k W[k, d] * E[k, q]      (PSUM accumulation over k)

3. MM1 runs as 3 fp8e4 DoubleRow matmuls (hi/lo error compensation:
   Kh Qh + Kh Ql + Kl Qh), 0.5 cycles/row with a 256-deep contraction:
   25% fewer PE cycles than one bf16 pass, ~0.6% score error.

4. PSUM: 2-bank [128,1024] score tiles (double-buffered, 4 banks) + all
   four [128,512] OT accumulators (4 banks) live through the whole
   k-block loop, LAG blocks behind the softmax pipeline -- no serial
   matmul phase-2. Chains drain one at a time at the end so each copy
   (DVE, ->fp16) + store overlaps the next chain's matmuls.

Precision: fp8-hilo scores (~0.6%), exact exp on ACT, fp16 E/W, fp32
PSUM accumulation, fp16 output staging -> rel err ~4.7e-3 (gate 2e-2).
"""

import numpy as np
import ml_dtypes

B, N, D = 8, 2048, 256
NCORES = 8
P = 128          # partitions
NU = 1024        # compacted query columns per core (device-fixed)
KB = N // P      # 16 k-blocks
NCH = NU // 512  # 2 output chunks of 512 (one PSUM bank each)
DT = D // P      # 2 d-tiles (contraction over d = 256)
LAG = 2          # k-blocks of slack before interleaved matmul-2 consumes W

_cached = None


def _build():
    import concourse.bacc as bacc
    import concourse.mybir as mybir
    import concourse.tile as tile

    f32 = mybir.dt.float32
    bf16 = mybir.dt.bfloat16
    f16 = mybir.dt.float16
    f8 = mybir.dt.float8e4
    DR = mybir.MatmulPerfMode.DoubleRow
    EXP = mybir.ActivationFunctionType.Exp
    ADD = mybir.AluOpType.add

    nc = bacc.Bacc()
    # hi/lo fp8 pairs, concatenated on a leading axis: [2(hl), D, cols]
    kt8 = nc.dram_tensor("kt8", [2, D, N], f8, kind="ExternalInput")
    qt8 = nc.dram_tensor("qt8", [2, D, NU], f8, kind="ExternalInput")
    v = nc.dram_tensor("v", [N, D], bf16, kind="ExternalInput")
    cadj = nc.dram_tensor("cadj", [P, KB], f32, kind="ExternalInput")
    ot = nc.dram_tensor("ot", [D, NU], f16, kind="ExternalOutput")
    cout = nc.dram_tensor("cout", [P, KB], f32, kind="ExternalOutput")

    with tile.TileContext(nc) as tc:
        with (
            tc.tile_pool(name="const", bufs=1) as constp,
            tc.tile_pool(name="epool", bufs=1) as epool,
            tc.tile_pool(name="cpool", bufs=1) as cpool,
            tc.tile_pool(name="outp", bufs=4) as outp,
            # all 4 OT accumulators live for the whole kernel (banks 0-3)
            tc.tile_pool(name="psA", bufs=1, space="PSUM") as psA,
        ):
            # SBUF inputs: [d_part, hl, d_tile, cols] so each DoubleRow
            # matmul slices a [128, 2, x] 3D AP (contraction d = part+tile).
            kt_sb = constp.tile([P, 2, DT, N], f8, name="kt_sb")
            qt_sb = constp.tile([P, 2, DT, NU], f8, name="qt_sb")
            v_sb = constp.tile([P, KB, D], bf16, name="v_sb")
            w_sb = constp.tile([P, KB, D], f16, name="w_sb")
            cadj_sb = constp.tile([P, KB], f32, name="cadj_sb")
            ctile = cpool.tile([P, KB], f32, name="ctile")
            rctile = cpool.tile([P, KB], f32, name="rctile")

            def dram_hl(t, cols0, cols1):
                # [2, D, x] DRAM slice -> [128, 2(hl), DT, x]
                return t[:, :, cols0:cols1].rearrange(
                    "h (t p) c -> p h t c", p=P)

            # Ordered by first consumption. fp8 chunks narrower than 512
            # cols pay a 2x DMA latency penalty (sub-512B contiguous runs),
            # so kt/qt move in >=512-col pieces: kt[0:1024] covers kb0-7,
            # qt[0:512] is exactly chunk 0 of every k-block.
            nc.sync.dma_start(kt_sb[:, :, :, 0:1024], dram_hl(kt8, 0, 1024))
            nc.scalar.dma_start(qt_sb[:, :, :, 0:512], dram_hl(qt8, 0, 512))
            nc.sync.dma_start(qt_sb[:, :, :, 512:NU], dram_hl(qt8, 512, NU))
            nc.scalar.dma_start(kt_sb[:, :, :, 1024:N],
                                dram_hl(kt8, 1024, N))
            nc.sync.dma_start(
                v_sb[:, 0:8, :],
                v[0:8 * P, :].rearrange("(s p) d -> p s d", p=P))
            nc.scalar.dma_start(cadj_sb[:], cadj[:, :])
            nc.sync.dma_start(
                v_sb[:, 8:KB, :],
                v[8 * P:KB * P, :].rearrange("(s p) d -> p s d", p=P))

            accA = [[psA.tile([P, 512], f32, name=f"accA{dh}_{ch}")
                     for ch in range(NCH)] for dh in range(DT)]

            # Warm the PE (p-state ramp) during the initial DMA wait; the
            # garbage lands in accA[0][0] and is cleared by its first
            # start=True accumulation.
            zs = constp.tile([P, 256], f8, name="zs")
            nc.vector.memset(zs[:], 0.0)
            for _ in range(11):
                nc.tensor.matmul(accA[0][0][:, 0:256], zs[:, 0:P],
                                 zs[:], start=True, stop=True)

            e_all = [None] * KB

            def mm2_step(dh, ch, kb):
                nc.tensor.matmul(
                    accA[dh][ch][:],
                    w_sb[:, kb, dh * P:(dh + 1) * P],
                    e_all[kb][:, ch * 512:(ch + 1) * 512],
                    start=(kb == 0),
                    stop=(kb == KB - 1),
                )

            with tc.tile_pool(name="psS", bufs=2, space="PSUM") as psS:
                for kb in range(KB):
                    st = psS.tile([P, NU], f32, name="st")
                    kw = (slice(None), slice(None))  # placeholder
                    for ch in range(NCH):
                        cs = slice(ch * 512, (ch + 1) * 512)
                        ks = slice(kb * P, (kb + 1) * P)
                        # hi*hi, hi*lo, lo*hi fp8 DoubleRow accumulation
                        for i, (hk, hq) in enumerate(((0, 0), (0, 1), (1, 0))):
                            nc.tensor.matmul(
                                st[:, cs],
                                kt_sb[:, hk, :, ks],
                                qt_sb[:, hq, :, cs],
                                start=(i == 0),
                                stop=(i == 2),
                                perf_mode=DR,
                            )
                    e_kb = epool.tile([P, NU], f16, name=f"e{kb}")
                    # c[k] = sum_q E, accumulated by the ACT engine during exp
                    nc.scalar.activation(e_kb[:], st[:], EXP, scale=1.0 / 16.0,
                                         accum_out=ctile[:, kb:kb + 1])
                    nc.vector.tensor_tensor(
                        ctile[:, kb:kb + 1], ctile[:, kb:kb + 1],
                        cadj_sb[:, kb:kb + 1], ADD)
                    nc.vector.reciprocal(rctile[:, kb:kb + 1],
                                         ctile[:, kb:kb + 1])
                    nc.vector.tensor_scalar_mul(
                        w_sb[:, kb, :], v_sb[:, kb, :], rctile[:, kb:kb + 1])
                    e_all[kb] = e_kb
                    if kb >= LAG:
                        for dh in range(DT):
                            for ch in range(NCH):
                                mm2_step(dh, ch, kb - LAG)

                nc.sync.dma_start(cout[:, :], ctile[:])
                # Drain chain-by-chain; copies split across DVE and ACT so
                # two run in parallel while later chains finish on the PE.
                # One output DMA per d-half (fewer serialized HWDGE preps).
                o_sb = [outp.tile([P, NU], f16, name=f"o_sb{dh}")
                        for dh in range(DT)]
                for dh in range(DT):
                    for ch in range(NCH):
                        for kb in range(KB - LAG, KB):
                            mm2_step(dh, ch, kb)
                        dst = o_sb[dh][:, ch * 512:(ch + 1) * 512]
                        if ch % 2 == 0:
                            nc.vector.tensor_copy(dst, accA[dh][ch][:])
                        else:
                            nc.scalar.copy(dst, accA[dh][ch][:])
                    eng = nc.sync if dh == 0 else nc.scalar
                    eng.dma_start(ot[dh * P:(dh + 1) * P, :], o_sb[dh][:])

    nc.compile()
    return nc


def _get_nc():
    global _cached
    if _cached is None:
        _cached = _build()
    return _cached


def _hilo8(x):
    """fp8e4m3 hi/lo decomposition along a new leading axis."""
    f8n = ml_dtypes.float8_e4m3
    hi = x.astype(f8n)
    lo = (x - hi.astype(np.float32)).astype(f8n)
    return np.stack([hi, lo], axis=0)


def kernel(key, query, value, mask):
    from concourse.bass_utils import run_bass_kernel_spmd

    nc = _get_nc()
    bf = ml_dtypes.bfloat16
    key = np.asarray(key, dtype=np.float32)
    query = np.asarray(query, dtype=np.float32)
    value = np.asarray(value, dtype=np.float32)
    mask = np.asarray(mask)

    in_maps = []
    host = []  # per-batch host-side state for postprocessing
    for b in range(B):
        m = mask[b, 0].astype(bool)
        idx = np.nonzero(m)[0]
        dev_idx = idx[:NU]
        ex_idx = idx[NU:]
        nd = len(dev_idx)
        npad = NU - nd

        qdev = np.zeros((NU, D), np.float32)
        qdev[:nd] = query[b][dev_idx]
        # cextra[k]: contribution of host-handled overflow queries to the
        # softmax normalizer. Pad columns contribute exp(0)=1 each.
        if len(ex_idx):
            s_ex = (key[b] @ query[b][ex_idx].T) / 16.0   # (N, ne)
            e_ex = np.exp(s_ex)
            cextra = e_ex.sum(axis=1)
        else:
            e_ex = None
            cextra = np.zeros(N, np.float32)
        cadj = (cextra - float(npad)).astype(np.float32)

        in_maps.append({
            "kt8": _hilo8(np.ascontiguousarray(key[b].T)),
            "qt8": _hilo8(np.ascontiguousarray(qdev.T)),
            "v": np.ascontiguousarray(value[b]).astype(bf),
            "cadj": np.ascontiguousarray(cadj.reshape(KB, P).T),
        })
        host.append((dev_idx, ex_idx, nd, e_ex))

    res = None
    for attempt in range(4):
        try:
            res = run_bass_kernel_spmd(nc, in_maps, core_ids=list(range(NCORES)))
            break
        except Exception:
            # Transient "accelerator device unrecoverable" states wedge the
            # PJRT client but not the device: tear down the backend and retry.
            if attempt == 3:
                raise
            import time
            time.sleep(10 * (attempt + 1))
            try:
                import jax.extend.backend as _jb
                _jb.clear_backends()
                import jax
                jax.clear_caches()
            except Exception:
                pass

    out = np.zeros((B, N, D), np.float32)
    for b in range(B):
        dev_idx, ex_idx, nd, e_ex = host[b]
        otb = res.results[b]["ot"].astype(np.float32)   # (D, NU)
        out[b][dev_idx] = otb.T[:nd]
        if len(ex_idx):
            c = res.results[b]["cout"].T.reshape(N)     # (N,) corrected c
            a_ex = e_ex / c[:, None]                    # (N, ne)
            out[b][ex_idx] = a_ex.T @ value[b]
    return out


# revision 15
# speedup vs baseline: 1.0358x; 1.0358x over previous
"""Trainium2 Bass kernel for nn_Attention_326417514823.

Per-batch computation (B=8, N=2048, D=256), one batch per NeuronCore:
    S = Q @ K.T / sqrt(D)                  (N x N)
    S[q, :] = -1e9 where mask[q] == 0      (row masking by query index)
    A = softmax(S, axis=0)                 (normalize over q, per column k)
    A[q, :] = 0 where mask[q] == 0
    O = A @ V                              (N x D)

Key restructuring vs a dense kernel:

1. HOST-SIDE QUERY COMPACTION. The softmax axis is q, and masked queries
   contribute nothing: their output rows are zero and they are excluded
   from every softmax sum. The host packs the first <=1024 unmasked
   queries into a fixed [256, 1024] device tile (pad columns are zero ->
   scores 0 -> E=1, subtracted out of the normalizer via a host-provided
   per-k correction).  Overflow queries (n_u > 1024, a ~2% tail) are
   handled exactly on the host using the device-returned normalizers
   c[k]: O_excess = (exp(S_excess)/c).T @ V.  This halves all on-device
   work (PE, exp, DMA).

2. TRANSPOSED LAYOUT. ST[k, q] = KT.T @ QT with d on partitions, so the
   softmax reduction runs along the free axis and neither matmul needs an
   on-chip transpose:
     E[k, q]  = exp(ST/16)                   (fp16, ScalarE, fused accum c)
     W[k, :]  = V[k, :] * (1/c[k])           (fp16, DVE)
     OT[d, q] = sum_k W[k, d] * E[k, q]      (PSUM accumulation over k)

3. MM1 runs as 3 fp8e4 DoubleRow matmuls (hi/lo error compensation:
   Kh Qh + Kh Ql + Kl Qh), 0.5 cycles/row with a 256-deep contraction:
   25% fewer PE cycles than one bf16 pass, ~0.6% score error.

4. PSUM: 2-bank [128,1024] score tiles (double-buffered, 4 banks) + all
   four [128,512] OT accumulators (4 banks) live through the whole
   k-block loop, LAG blocks behind the softmax pipeline -- no serial
   matmul phase-2. Chains drain one at a time at the end so each copy
   (DVE, ->fp16) + store overlaps the next chain's matmuls.

Precision: fp8-hilo scores (~0.6%), exact exp on ACT, fp16 E/W, fp32
PSUM accumulation, fp16 output staging -> rel err ~4.7e-3 (gate 2e-2).
"""

import numpy as np
import ml_dtypes

B, N, D = 8, 2048, 256
NCORES = 8
P = 128          # partitions
NU = 1024        # compacted query columns per core (device-fixed)
KB = N // P      # 16 k-blocks
NCH = NU // 512  # 2 output chunks of 512 (one PSUM bank each)
DT = D // P      # 2 d-tiles (contraction over d = 256)
LAG = 2          # k-blocks of slack before interleaved matmul-2 consumes W

_cached = None


def _build():
    import concourse.bacc as bacc
    import concourse.mybir as mybir
    import concourse.tile as tile

    f32 = mybir.dt.float32
    bf16 = mybir.dt.bfloat16
    f16 = mybir.dt.float16
    f8 = mybir.dt.float8e4
    DR = mybir.MatmulPerfMode.DoubleRow
    EXP = mybir.ActivationFunctionType.Exp
    ADD = mybir.AluOpType.add

    nc = bacc.Bacc()
    # hi/lo fp8 pairs, concatenated on a leading axis: [2(hl), D, cols]
    kt8 = nc.dram_tensor("kt8", [2, D, N], f8, kind="ExternalInput")
    qt8 = nc.dram_tensor("qt8", [2, D, NU], f8, kind="ExternalInput")
    v = nc.dram_tensor("v", [N, D], bf16, kind="ExternalInput")
    cadj = nc.dram_tensor("cadj", [P, KB], f32, kind="ExternalInput")
    ot = nc.dram_tensor("ot", [D, NU], f16, kind="ExternalOutput")
    cout = nc.dram_tensor("cout", [P, KB], f32, kind="ExternalOutput")

    with tile.TileContext(nc) as tc:
        with (
            tc.tile_pool(name="const", bufs=1) as constp,
            tc.tile_pool(name="epool", bufs=1) as epool,
            tc.tile_pool(name="cpool", bufs=1) as cpool,
            tc.tile_pool(name="outp", bufs=4) as outp,
            # all 4 OT accumulators live for the whole kernel (banks 0-3)
            tc.tile_pool(name="psA", bufs=1, space="PSUM") as psA,
        ):
            # SBUF inputs: [d_part, hl, d_tile, cols] so each DoubleRow
            # matmul slices a [128, 2, x] 3D AP (contraction d = part+tile).
            kt_sb = constp.tile([P, 2, DT, N], f8, name="kt_sb")
            qt_sb = constp.tile([P, 2, DT, NU], f8, name="qt_sb")
            v_sb = constp.tile([P, KB, D], bf16, name="v_sb")
            w_sb = constp.tile([P, KB, D], f16, name="w_sb")
            cadj_sb = constp.tile([P, KB], f32, name="cadj_sb")
            ctile = cpool.tile([P, KB], f32, name="ctile")
            rctile = cpool.tile([P, KB], f32, name="rctile")

            def dram_hl(t, cols0, cols1):
                # [2, D, x] DRAM slice -> [128, 2(hl), DT, x]
                return t[:, :, cols0:cols1].rearrange(
                    "h (t p) c -> p h t c", p=P)

            # Ordered by first consumption. fp8 chunks narrower than 512
            # cols pay a 2x DMA latency penalty (sub-512B contiguous runs),
            # so kt/qt move in >=512-col pieces: kt[0:1024] covers kb0-7,
            # qt[0:512] is exactly chunk 0 of every k-block.
            nc.sync.dma_start(kt_sb[:, :, :, 0:512], dram_hl(kt8, 0, 512))
            nc.scalar.dma_start(qt_sb[:, :, :, 0:512], dram_hl(qt8, 0, 512))
            nc.sync.dma_start(qt_sb[:, :, :, 512:NU], dram_hl(qt8, 512, NU))
            nc.scalar.dma_start(kt_sb[:, :, :, 512:1024],
                                dram_hl(kt8, 512, 1024))
            nc.sync.dma_start(cadj_sb[:], cadj[:, :])
            nc.scalar.dma_start(
                v_sb[:, 0:8, :],
                v[0:8 * P, :].rearrange("(s p) d -> p s d", p=P))
            nc.sync.dma_start(kt_sb[:, :, :, 1024:N],
                              dram_hl(kt8, 1024, N))
            nc.scalar.dma_start(
                v_sb[:, 8:KB, :],
                v[8 * P:KB * P, :].rearrange("(s p) d -> p s d", p=P))

            accA = [[psA.tile([P, 512], f32, name=f"accA{dh}_{ch}")
                     for ch in range(NCH)] for dh in range(DT)]

            # Warm the PE (p-state ramp) during the initial DMA wait; the
            # garbage lands in accA[0][0] and is cleared by its first
            # start=True accumulation.
            zs = constp.tile([P, 256], f8, name="zs")
            nc.vector.memset(zs[:], 0.0)
            # enough dummies to stay busy until the first real matmul's
            # operands land -- a PE idle gap resets the p-state ramp
            for _ in range(15):
                nc.tensor.matmul(accA[0][0][:, 0:256], zs[:, 0:P],
                                 zs[:], start=True, stop=True)

            e_all = [None] * KB

            def mm2_step(dh, ch, kb):
                nc.tensor.matmul(
                    accA[dh][ch][:],
                    w_sb[:, kb, dh * P:(dh + 1) * P],
                    e_all[kb][:, ch * 512:(ch + 1) * 512],
                    start=(kb == 0),
                    stop=(kb == KB - 1),
                )

            with tc.tile_pool(name="psS", bufs=2, space="PSUM") as psS:
                for kb in range(KB):
                    st = psS.tile([P, NU], f32, name="st")
                    kw = (slice(None), slice(None))  # placeholder
                    for ch in range(NCH):
                        cs = slice(ch * 512, (ch + 1) * 512)
                        ks = slice(kb * P, (kb + 1) * P)
                        # hi*hi, hi*lo, lo*hi fp8 DoubleRow accumulation
                        for i, (hk, hq) in enumerate(((0, 0), (0, 1), (1, 0))):
                            nc.tensor.matmul(
                                st[:, cs],
                                kt_sb[:, hk, :, ks],
                                qt_sb[:, hq, :, cs],
                                start=(i == 0),
                                stop=(i == 2),
                                perf_mode=DR,
                            )
                    e_kb = epool.tile([P, NU], f16, name=f"e{kb}")
                    # c[k] = sum_q E, accumulated by the ACT engine during exp
                    nc.scalar.activation(e_kb[:], st[:], EXP, scale=1.0 / 16.0,
                                         accum_out=ctile[:, kb:kb + 1])
                    nc.vector.tensor_tensor(
                        ctile[:, kb:kb + 1], ctile[:, kb:kb + 1],
                        cadj_sb[:, kb:kb + 1], ADD)
                    nc.vector.reciprocal(rctile[:, kb:kb + 1],
                                         ctile[:, kb:kb + 1])
                    nc.vector.tensor_scalar_mul(
                        w_sb[:, kb, :], v_sb[:, kb, :], rctile[:, kb:kb + 1])
                    e_all[kb] = e_kb
                    if kb >= LAG:
                        for dh in range(DT):
                            for ch in range(NCH):
                                mm2_step(dh, ch, kb - LAG)

                nc.sync.dma_start(cout[:, :], ctile[:])
                # Drain chain-by-chain; copies split across DVE and ACT so
                # two run in parallel while later chains finish on the PE.
                # One output DMA per d-half (fewer serialized HWDGE preps).
                o_sb = [outp.tile([P, NU], f16, name=f"o_sb{dh}")
                        for dh in range(DT)]
                for dh in range(DT):
                    for ch in range(NCH):
                        for kb in range(KB - LAG, KB):
                            mm2_step(dh, ch, kb)
                        dst = o_sb[dh][:, ch * 512:(ch + 1) * 512]
                        if ch % 2 == 0:
                            nc.vector.tensor_copy(dst, accA[dh][ch][:])
                        else:
                            nc.scalar.copy(dst, accA[dh][ch][:])
                    eng = nc.sync if dh == 0 else nc.scalar
                    eng.dma_start(ot[dh * P:(dh + 1) * P, :], o_sb[dh][:])

    nc.compile()
    return nc


def _get_nc():
    global _cached
    if _cached is None:
        _cached = _build()
    return _cached


def _hilo8(x):
    """fp8e4m3 hi/lo decomposition along a new leading axis."""
    f8n = ml_dtypes.float8_e4m3
    hi = x.astype(f8n)
    lo = (x - hi.astype(np.float32)).astype(f8n)
    return np.stack([hi, lo], axis=0)


def kernel(key, query, value, mask):
    from concourse.bass_utils import run_bass_kernel_spmd

    nc = _get_nc()
    bf = ml_dtypes.bfloat16
    key = np.asarray(key, dtype=np.float32)
    query = np.asarray(query, dtype=np.float32)
    value = np.asarray(value, dtype=np.float32)
    mask = np.asarray(mask)

    in_maps = []
    host = []  # per-batch host-side state for postprocessing
    for b in range(B):
        m = mask[b, 0].astype(bool)
        idx = np.nonzero(m)[0]
        dev_idx = idx[:NU]
        ex_idx = idx[NU:]
        nd = len(dev_idx)
        npad = NU - nd

        qdev = np.zeros((NU, D), np.float32)
        qdev[:nd] = query[b][dev_idx]
        # cextra[k]: contribution of host-handled overflow queries to the
        # softmax normalizer. Pad columns contribute exp(0)=1 each.
        if len(ex_idx):
            s_ex = (key[b] @ query[b][ex_idx].T) / 16.0   # (N, ne)
            e_ex = np.exp(s_ex)
            cextra = e_ex.sum(axis=1)
        else:
            e_ex = None
            cextra = np.zeros(N, np.float32)
        cadj = (cextra - float(npad)).astype(np.float32)

        in_maps.append({
            "kt8": _hilo8(np.ascontiguousarray(key[b].T)),
            "qt8": _hilo8(np.ascontiguousarray(qdev.T)),
            "v": np.ascontiguousarray(value[b]).astype(bf),
            "cadj": np.ascontiguousarray(cadj.reshape(KB, P).T),
        })
        host.append((dev_idx, ex_idx, nd, e_ex))

    res = None
    for attempt in range(4):
        try:
            res = run_bass_kernel_spmd(nc, in_maps, core_ids=list(range(NCORES)))
            break
        except Exception:
            # Transient "accelerator device unrecoverable" states wedge the
            # PJRT client but not the device: tear down the backend and retry.
            if attempt == 3:
                raise
            import time
            time.sleep(10 * (attempt + 1))
            try:
                import jax.extend.backend as _jb
                _jb.clear_backends()
                import jax
                jax.clear_caches()
            except Exception:
                pass

    out = np.zeros((B, N, D), np.float32)
    for b in range(B):
        dev_idx, ex_idx, nd, e_ex = host[b]
        otb = res.results[b]["ot"].astype(np.float32)   # (D, NU)
        out[b][dev_idx] = otb.T[:nd]
        if len(ex_idx):
            c = res.results[b]["cout"].T.reshape(N)     # (N,) corrected c
            a_ex = e_ex / c[:, None]                    # (N, ne)
            out[b][ex_idx] = a_ex.T @ value[b]
    return out
